# revision 17
# baseline (speedup 1.0000x reference)
"""GCN inference kernel (y = D^-1/2 A D^-1/2 (x @ W.T)) on 8 Trainium2 NeuronCores.

Strategy (full inputs in, full output out; sharded internally):
  - Destination nodes are sharded across the 8 cores (12500 dsts each);
    edges are owned by the core that owns their dst, so the segment-sum is
    core-local (per the sharding hint).
  - Phase A (sharded): each core computes the scaled projection table
    h~[n] = dinv[n] * (x[n] @ W.T) for its 12800-node shard with PE matmuls
    (bf16), writing bf16 rows padded to 256B (SWDGE gather elem_size must
    be a multiple of 256B). An AllGather assembles the full table in DRAM.
  - Phase B (per core): ONE SWDGE dma_gather per (superwindow, bucket)
    streams h~[src] rows for the core's dst-sorted edge list into SBUF
    (~4.5k descriptors per call — the per-call Pool-engine desc-gen
    overhead was the baseline bottleneck at 392 small calls); a one-hot
    selection matrix B (one DVE is_equal per call, bf16, dst-local ids vs
    an iota row) turns the segment-sum into PE matmuls accumulated in PSUM
    per 128-dst tile; a final per-dst dinv scale lands y.
  - Padding slots gather row 0 of the bucket (a real, finite row) and
    carry dstl=-1 so their one-hot column is zero: no runtime descriptor
    counts (reg_load), no pool-buffer memsets.
  - All data-dependent structure (edge sort, padding, gather indices,
    one-hot ids, uniform per-core slice schedule) is prepared host-side in
    numpy; the device program is identical on all 8 cores (SPMD), only the
    per-core input arrays differ.
"""

import contextlib
import math
from dataclasses import dataclass, field

import ml_dtypes
import numpy as np

import concourse.bacc as bacc
import concourse.bass as bass
import concourse.mybir as mybir
import concourse.tile as tile
from concourse import library_config
from concourse.bass_utils import run_bass_kernel_spmd

P = 128  # SBUF partitions
FIN = 128
FOUT = 64
TROW = 2 * FOUT  # table row: 64 bf16 data + 64 bf16 pad = 256B


@dataclass
class Prm:
    N: int = 100000  # nodes
    C: int = 8  # cores
    WG: int = 640  # nodes per phase-A write group
    GQ: int = 5  # write groups per quarter (= gather bucket)
    SWD: int = 512  # dst nodes per superwindow (TPSW * P)
    GBUFS: int = 6  # gather/one-hot pool depth (in calls)
    J: int = field(init=False)
    NS: int = field(init=False)  # dst shard size per core
    N2: int = field(init=False)  # padded node count (multiple of C*WG)
    NG: int = field(init=False)  # phase-A write groups
    NGpc: int = field(init=False)  # phase-A write groups per core
    NBK: int = field(init=False)  # gather buckets (int16 idx limit)
    SHN: int = field(init=False)  # nodes per phase-A shard
    QN: int = field(init=False)  # nodes per (core, quarter)
    BKCAP: int = field(init=False)  # table rows per gather bucket
    TBLR: int = field(init=False)  # total table rows
    TPSW: int = field(init=False)  # dst tiles per superwindow
    NSW: int = field(init=False)  # superwindows per core

    def __post_init__(self):
        assert self.WG % P == 0
        assert self.SWD % P == 0
        assert self.N % self.C == 0
        self.J = self.WG // P
        self.NS = self.N // self.C
        blk = self.C * self.WG
        self.N2 = ((self.N + blk - 1) // blk) * blk
        self.NG = self.N2 // self.WG
        self.NGpc = self.NG // self.C
        assert self.NGpc % self.GQ == 0
        self.NBK = self.NGpc // self.GQ
        self.SHN = self.N2 // self.C
        self.QN = self.GQ * self.WG
        self.BKCAP = self.C * self.QN
        assert self.BKCAP <= 32767
        self.TBLR = self.N2
        self.TPSW = self.SWD // P
        self.NSW = (self.NS + self.SWD - 1) // self.SWD


def _rmap(prm, n):
    """node id -> table row, quarter-major layout matching the single
    AllGather's concatenation of per-core shards (4 small per-quarter
    collectives measured SLOWER: ~25us fixed overhead each, serialized on
    the CC cores, so the last bucket landed at 247us vs 176us)."""
    c = n // prm.SHN
    i2 = n % prm.SHN
    k = i2 // prm.QN
    i = i2 % prm.QN
    wrap = prm.WG * (i // prm.WG) + prm.J * (i % P) + (i % prm.WG) // P
    return k * prm.BKCAP + c * prm.QN + wrap


def _wrap_idx(vals16):
    """[K] int16 (K % 128 == 0) -> [128, K//16] wrapped+replicated layout."""
    k = vals16.shape[0]
    w16 = vals16.reshape(k // 16, 16).T  # [16, K/16]
    return np.tile(w16, (8, 1))  # [128, K/16]


@dataclass
class CallMeta:
    sw: int
    bk: int
    S: int  # slices in this call (one dma_gather per call)
    icol: int  # column offset into gidx array (8 * slice offset)
    scol: int  # column offset into dstl array (slice offset)


def _schedule(prm, n_sl_u):
    """Uniform (core-independent) schedule: one gather call per (sw, bk)
    covering all TPSW dst tiles (t-major slice layout inside the call).
    Matmuls are emitted per dst-tile PAIR (two PSUM banks, double-buffered),
    bucket-major within a pair; each (sw, t) accumulates into its own PSUM
    tensor (start on its first mm, stop on last).

    Returns (calls, mms_by_sw, icol_total, scol_total).
    mms_by_sw[sw] = list of (bk, t, csl, start, stop); lhsT/rhs come from
    call (sw, bk) call-local slice csl.
    """
    calls = []
    mms_by_sw = []
    icol = 0
    scol = 0
    for sw in range(prm.NSW):
        for bk in range(prm.NBK):
            S = int(sum(n_sl_u[sw][bk][t] for t in range(prm.TPSW)))
            if S == 0:
                continue
            calls.append(CallMeta(sw, bk, S, icol, scol))
            icol += 8 * S
            scol += S
        tot = [
            sum(int(n_sl_u[sw][bk][t]) for bk in range(prm.NBK))
            for t in range(prm.TPSW)
        ]
        seen = [0] * prm.TPSW
        # bucket-major: matmuls for bucket bk start as soon as ITS gather and
        # one-hot land (all TPSW accumulators open concurrently in separate
        # PSUM banks) — no wait for the sw's last bucket
        mms = []
        for bk in range(prm.NBK):
            for t in range(prm.TPSW):
                off = int(sum(n_sl_u[sw][bk][tt] for tt in range(t)))
                for sl in range(int(n_sl_u[sw][bk][t])):
                    mms.append(
                        (bk, t, off + sl, seen[t] == 0, seen[t] == tot[t] - 1)
                    )
                    seen[t] += 1
        mms_by_sw.append(mms)
    return calls, mms_by_sw, icol, scol


def _host_prep(x, edge_index, W, prm):
    N, C, NS = prm.N, prm.C, prm.NS
    src = np.asarray(edge_index[0], dtype=np.int64).astype(np.int32)
    dst = np.asarray(edge_index[1], dtype=np.int64).astype(np.int32)
    x = np.asarray(x, dtype=np.float32)
    W = np.asarray(W, dtype=np.float32)

    deg = np.bincount(dst, minlength=N).astype(np.float64)
    dinv = np.where(deg > 0, 1.0 / np.sqrt(np.maximum(deg, 1.0)), 0.0).astype(
        np.float32
    )

    # gather-order node map
    r_of = _rmap(prm, np.arange(N, dtype=np.int64)).astype(np.int64)
    bk_of = (r_of // prm.BKCAP).astype(np.int32)
    rel_of = (r_of % prm.BKCAP).astype(np.int16)

    # per-edge attributes
    core_e = dst // NS
    edl = dst - core_e * NS
    sw_e = edl // prm.SWD
    t_e = (edl % prm.SWD) // P
    q_e = (edl % P).astype(np.float32)
    bk_e = bk_of[src]
    rel_e = rel_of[src]

    # per-core cell structure; edges sorted by table row within each cell
    # (HBM page locality for the gather stream)
    ncell = prm.NSW * prm.NBK * prm.TPSW
    counts = np.zeros((C, ncell), dtype=np.int64)
    percore = []
    for c in range(C):
        m = core_e == c
        order = np.lexsort((rel_e[m], t_e[m], bk_e[m], sw_e[m]))
        cell = (sw_e[m] * prm.NBK + bk_e[m]) * prm.TPSW + t_e[m]
        counts[c] = np.bincount(cell, minlength=ncell)
        percore.append(
            {
                "rel": rel_e[m][order],
                "q": q_e[m][order],
                "cell": cell[order],
            }
        )

    # uniform slice counts; ensure every in-range (sw, t) has >= 1 slice
    # somewhere so its PSUM accumulation group opens and closes
    n_sl_u = np.zeros((prm.NSW, prm.NBK, prm.TPSW), dtype=np.int64)
    cmax = counts.max(axis=0).reshape(prm.NSW, prm.NBK, prm.TPSW)
    n_sl_u[:] = (cmax + P - 1) // P
    for sw in range(prm.NSW):
        ntile = min(prm.TPSW, max(0, -(-(NS - sw * prm.SWD) // P)))
        for t in range(ntile):
            if n_sl_u[sw, :, t].sum() == 0:
                n_sl_u[sw, 0, t] = 1

    calls, mms_by_sw, icols, scols = _schedule(prm, n_sl_u)

    # slot offset (in slices) of each cell in the uniform stream; cells are
    # ordered (sw, bk, t) with t fastest, which matches the t-major layout
    # inside each (sw, bk) call
    cell_sl = n_sl_u.reshape(ncell)
    cell_off = np.zeros(ncell, dtype=np.int64)
    np.cumsum(cell_sl[:-1], out=cell_off[1:])
    S_total = int(cell_sl.sum())

    # fill per-core gather-index / dst-local arrays; padding slots gather
    # row 0 (real data) with dstl=-1 (zero one-hot column)
    gidx_all = np.zeros((C, P, icols), dtype=np.int16)
    dstl_all = np.full((C, P, scols), -1.0, dtype=ml_dtypes.bfloat16)
    for c in range(C):
        pc = percore[c]
        ne = pc["cell"].shape[0]
        cc = counts[c]
        starts = np.zeros(ncell, dtype=np.int64)
        np.cumsum(cc[:-1], out=starts[1:])
        rank = np.arange(ne, dtype=np.int64) - starts[pc["cell"]]
        pos = cell_off[pc["cell"]] * P + rank  # global slot position
        vals = np.zeros(S_total * P, dtype=np.int16)  # pad -> row 0
        dvals = np.full(S_total * P, -1.0, dtype=np.float32)
        vals[pos] = pc["rel"]
        dvals[pos] = pc["q"]
        for cm in calls:
            sl0 = cm.scol
            seg = vals[sl0 * P : (sl0 + cm.S) * P]
            gidx_all[c, :, cm.icol : cm.icol + 8 * cm.S] = _wrap_idx(seg)
            dstl_all[c, :, cm.scol : cm.scol + cm.S] = (
                dvals[sl0 * P : (sl0 + cm.S) * P].reshape(cm.S, P).T
            )

    # phase-A inputs
    xT = np.zeros((FIN, prm.N2), dtype=ml_dtypes.bfloat16)
    xT[:, :N] = x.T.astype(ml_dtypes.bfloat16)
    WT = np.ascontiguousarray(W.T).astype(ml_dtypes.bfloat16)  # [FIN, FOUT]
    dpad = np.zeros(prm.N2, dtype=np.float32)
    dpad[:N] = dinv
    iota = np.broadcast_to(
        np.arange(P, dtype=ml_dtypes.bfloat16)[None, :], (P, P)
    ).copy()
    dinvD = np.zeros((C, P, prm.NSW * prm.TPSW), dtype=np.float32)
    w_idx = np.arange(prm.NSW * prm.TPSW)
    for c in range(C):
        node = c * NS + w_idx[:, None] * P + np.arange(P)[None, :]
        ok = node < (c + 1) * NS
        dv = np.where(ok, dinv[np.minimum(node, N - 1)], 0.0)
        dinvD[c][np.arange(P)[None, :], w_idx[:, None]] = dv

    # phase-A shard for core c: the nodes whose table rows fall in its
    # AllGather output block [SHN*c, SHN*(c+1)) of the quarter-major layout:
    # quarter (c//2) of original node shards 4*(c%2) .. 4*(c%2)+3
    inputs = []
    i2 = np.arange(prm.SHN)
    for c in range(C):
        segs = [
            np.arange(prm.QN, dtype=np.int64)
            + (4 * (c % 2) + u) * prm.SHN
            + (c // 2) * prm.QN
            for u in range(C // 2)
        ]
        nodes = np.concatenate(segs)  # SHN nodes in TBSH write order
        assert nodes.shape[0] == prm.SHN
        seq = dpad[nodes]
        dinvA_c = np.zeros((P, prm.NGpc * prm.J), dtype=np.float32)
        dinvA_c[i2 % P, (i2 // prm.WG) * prm.J + (i2 % prm.WG) // P] = seq
        inputs.append(
            {
                "xT": np.ascontiguousarray(xT[:, nodes]),
                "WT": WT,
                "dinvA": dinvA_c,
                "iota": iota,
                "dinvD": dinvD[c],
                "gidx": gidx_all[c],
                "dstl": dstl_all[c],
            }
        )
    return inputs, calls, mms_by_sw


def _split_sync_waits(nc):
    """This env's walrus rejects >1 sync wait on some opcodes; keep 1 wait
    per instruction, moving extras onto preceding same-engine NOPs."""
    for bb in nc.main_func.blocks:
        insts = bb.instructions
        i = 0
        while i < len(insts):
            ins = insts[i]
            si = ins.sync_info
            if si is not None and si.on_wait is not None and len(si.on_wait) > 1:
                waits = list(si.on_wait)
                keep, extra = waits[-1:], waits[:-1]
                k = 0
                while extra:
                    chunk, extra = extra[:1], extra[1:]
                    nop = mybir.InstNoOp(name=f"{ins.name}-ws{k}", ins=[], outs=[])
                    nop.engine = ins.engine
                    nop.sync_info = mybir.SyncInfo(on_wait=chunk, on_update=[])
                    nc.register_instruction(nop)
                    insts.insert(i, nop)
                    i += 1
                    k += 1
                ins.sync_info = mybir.SyncInfo(
                    on_wait=keep, on_update=list(si.on_update or [])
                )
            i += 1


def _build_program(prm, calls, mms_by_sw, icols, scols):
    f32 = mybir.dt.float32
    bf16 = mybir.dt.bfloat16
    # 48KB/partition descriptor carveout -> 3072-desc SWDGE ring per queue
    # (default 16KB/1024 descs stalls desc-gen at transfer pace with almost
    # no pipeline buffer; the ring frees as transfers COMPLETE)
    nc = bacc.Bacc(
        "TRN2", num_swdge_queues=4, dynamic_dma_scratch_size=49152
    )

    NGpc = prm.NGpc
    xT = nc.declare_dram_parameter(
        "xT", [FIN, NGpc * prm.WG], bf16, isOutput=False
    )
    WT = nc.declare_dram_parameter("WT", [FIN, FOUT], bf16, isOutput=False)
    dinvA = nc.declare_dram_parameter(
        "dinvA", [P, NGpc * prm.J], f32, isOutput=False
    )
    iota = nc.declare_dram_parameter("iota", [P, P], bf16, isOutput=False)
    dinvD = nc.declare_dram_parameter(
        "dinvD", [P, prm.NSW * prm.TPSW], f32, isOutput=False
    )
    gidx = nc.declare_dram_parameter("gidx", [P, icols], mybir.dt.int16, isOutput=False)
    dstl = nc.declare_dram_parameter("dstl", [P, scols], bf16, isOutput=False)
    y = nc.declare_dram_parameter("y", [prm.NS, FOUT], f32, isOutput=True)
    # phase A is SHARDED: each core computes its 12800-row table shard, an
    # AllGather assembles the full table.
    TBSH = nc.dram_tensor("tbsh", [NGpc * prm.WG, TROW], bf16)
    TBLA = nc.dram_tensor(
        "tbla", [prm.TBLR, TROW], bf16, addr_space="Shared"
    )

    S_MAX = max(cm.S for cm in calls)
    calls_by_swbk = {}
    for cm in calls:
        calls_by_swbk[(cm.sw, cm.bk)] = cm

    # gidx/dstl are loaded in CHUNK_SW-superwindow mega-chunks (few large
    # HWDGE descriptors instead of one small load per call)
    CHUNK_SW = 5
    NCH = (prm.NSW + CHUNK_SW - 1) // CHUNK_SW
    ch_i0 = []  # (icol0, icol1, scol0, scol1) per chunk
    for ch in range(NCH):
        sws = [cm for cm in calls if ch * CHUNK_SW <= cm.sw < (ch + 1) * CHUNK_SW]
        i0 = min(cm.icol for cm in sws)
        i1 = max(cm.icol + 8 * cm.S for cm in sws)
        s0 = min(cm.scol for cm in sws)
        s1 = max(cm.scol + cm.S for cm in sws)
        ch_i0.append((i0, i1, s0, s1))
    ICH_MAX = max(i1 - i0 for i0, i1, _, _ in ch_i0)
    SCH_MAX = max(s1 - s0 for _, _, s0, s1 in ch_i0)

    with tile.TileContext(nc) as tc:
        _stk = contextlib.ExitStack()
        cpool = _stk.enter_context(tc.tile_pool(name="const", bufs=1))
        pa = _stk.enter_context(tc.tile_pool(name="pa", bufs=3))
        psa = _stk.enter_context(tc.tile_pool(name="psa", bufs=2, space="PSUM"))
        pidx = _stk.enter_context(tc.tile_pool(name="pidx", bufs=2))
        pg = _stk.enter_context(tc.tile_pool(name="pg", bufs=prm.GBUFS))
        pb = _stk.enter_context(tc.tile_pool(name="pb", bufs=prm.GBUFS))
        py = _stk.enter_context(tc.tile_pool(name="py", bufs=3))
        psb = _stk.enter_context(tc.tile_pool(name="psb", bufs=1, space="PSUM"))

        wt_sb = cpool.tile([FIN, FOUT], bf16, tag="wt")
        nc.sync.dma_start(out=wt_sb[:], in_=WT[:])
        dinvA_sb = cpool.tile([P, NGpc * prm.J], f32, tag="da")
        nc.sync.dma_start(out=dinvA_sb[:], in_=dinvA[:])
        iota_sb = cpool.tile([P, P], bf16, tag="io")
        nc.sync.dma_start(out=iota_sb[:], in_=iota[:])
        dinvD_sb = cpool.tile([P, prm.NSW * prm.TPSW], f32, tag="dd")
        nc.sync.dma_start(out=dinvD_sb[:], in_=dinvD[:])

        chunk_tiles = {}

        def load_chunk(ch):
            if ch >= NCH or ch in chunk_tiles:
                return
            i0, i1, s0, s1 = ch_i0[ch]
            idx_t = pidx.tile([P, ICH_MAX], mybir.dt.int16, tag="idx")
            nc.sync.dma_start(out=idx_t[:, : i1 - i0], in_=gidx[:, i0:i1])
            dst_t = pidx.tile([P, SCH_MAX], bf16, tag="dst")
            nc.sync.dma_start(out=dst_t[:, : s1 - s0], in_=dstl[:, s0:s1])
            chunk_tiles[ch] = (idx_t, dst_t)

        load_chunk(0)

        # ------- Phase A: build the table shard; per-quarter AllGather ----
        # collective q fires as soon as the 5 write groups of quarter q are
        # in TBSH, so bucket-q gathers pipeline with the rest of phase A
        for g in range(NGpc):
            xt = pa.tile([P, prm.WG], bf16, tag="xt")
            nc.sync.dma_start(
                out=xt[:], in_=xT[:, g * prm.WG : (g + 1) * prm.WG]
            )
            hps = psa.tile([P, prm.J * FOUT], f32, tag="hps")
            for j in range(prm.J):
                nc.tensor.matmul(
                    out=hps[:, j * FOUT : (j + 1) * FOUT],
                    lhsT=xt[:, j * P : (j + 1) * P],
                    rhs=wt_sb[:],
                    start=True,
                    stop=True,
                )
            tsb = pa.tile([P, prm.J, TROW], bf16, tag="tsb")
            if g < 3:  # zero pad cols once per buffer
                nc.vector.memset(tsb[:], 0.0)
            nc.vector.tensor_tensor(
                out=tsb[:, :, :FOUT],
                in0=hps[:].rearrange("p (j f) -> p j f", f=FOUT),
                in1=dinvA_sb[:, g * prm.J : (g + 1) * prm.J][
                    :, :, None
                ].to_broadcast([P, prm.J, FOUT]),
                op=mybir.AluOpType.mult,
            )
            base = prm.WG * g
            nc.sync.dma_start(
                out=TBSH[base : base + prm.WG, :].rearrange(
                    "(p j) f -> p j f", j=prm.J
                ),
                in_=tsb[:],
            )
        # assemble the full table from all cores' shards
        nc.gpsimd.collective_compute(
            "AllGather",
            mybir.AluOpType.bypass,
            replica_groups=[list(range(prm.C))],
            ins=[TBSH[:]],
            outs=[TBLA[:]],
        )
        TBL = [
            TBLA[k * prm.BKCAP : (k + 1) * prm.BKCAP, :]
            for k in range(prm.NBK)
        ]

        # ---------------- Phase B: gather + segment-sum ----------------
        def emit_call(cm, tiles):
            S = cm.S
            ch = cm.sw // CHUNK_SW
            idx_t, dst_t = chunk_tiles[ch]
            io = cm.icol - ch_i0[ch][0]
            so = cm.scol - ch_i0[ch][2]
            g_t = pg.tile([P, S_MAX, TROW], bf16, tag="g")
            nc.gpsimd.dma_gather(
                out_ap=g_t[:, :S, :],
                in_ap=TBL[cm.bk],
                idxs_ap=idx_t[:, io : io + 8 * S],
                num_idxs=S * P,
                num_idxs_reg=S * P,
                elem_size=TROW,
                single_packet=False,
                queue_num=cm.bk % 4,
            )
            b_t = pb.tile([P, S_MAX, P], bf16, tag="b")
            nc.vector.tensor_tensor(
                out=b_t[:, :S, :],
                in0=iota_sb[:, None, :].to_broadcast([P, S, P]),
                in1=dst_t[:, so : so + S][:, :, None].to_broadcast([P, S, P]),
                op=mybir.AluOpType.is_equal,
            )
            tiles[cm.bk] = (g_t, b_t)

        def emit_mms(sw, tiles):
            # accumulation groups must never share a PSUM bank
            # (start=True clears the whole bank) - one [P, FOUT] tile per
            # dst tile, all TPSW open concurrently in separate banks
            rows_sw = min(prm.SWD, prm.NS - sw * prm.SWD)
            nt = (rows_sw + P - 1) // P  # valid dst tiles this sw
            ysb = py.tile([P, prm.TPSW, FOUT], f32, tag="ysb")
            mms = mms_by_sw[sw]
            psum_t = {
                t: psb.tile([P, FOUT], f32, tag=f"acc{t}", name=f"acc{t}")
                for t in range(prm.TPSW)
            }
            closed = set()
            for bk, t, csl, st, sp in mms:
                g_t, b_t = tiles[bk]
                nc.tensor.matmul(
                    out=psum_t[t][:],
                    lhsT=b_t[:, csl, :],
                    rhs=g_t[:, csl, :FOUT],
                    start=st,
                    stop=sp,
                )
                if sp and t < nt:
                    # scale by dinv[dst] right after the group closes
                    w = sw * prm.TPSW + t
                    nc.scalar.activation(
                        out=ysb[:, t, :],
                        in_=psum_t[t][:],
                        func=mybir.ActivationFunctionType.Copy,
                        scale=dinvD_sb[:, w : w + 1],
                    )
                    closed.add(t)
            assert closed == set(range(nt)), (sw, closed, nt)
            for t in range(nt):
                rt = min(P, rows_sw - t * P)
                r0 = sw * prm.SWD + t * P
                nc.scalar.dma_start(
                    out=y[r0 : r0 + rt, :], in_=ysb[:rt, t, :]
                )

        for sw in range(prm.NSW):
            if sw % CHUNK_SW == 0:
                load_chunk(sw // CHUNK_SW + 1)  # prefetch next chunk
            tiles = {}
            for bk in range(prm.NBK):
                cm = calls_by_swbk.get((sw, bk))
                if cm is not None:
                    emit_call(cm, tiles)
            emit_mms(sw, tiles)
        _stk.close()

    nc.compile()
    _split_sync_waits(nc)
    return nc


def _get_program_and_prep(x, edge_index, W, prm):
    inputs, calls, mms_by_sw = _host_prep(x, edge_index, W, prm)
    icols = sum(8 * cm.S for cm in calls)
    scols = sum(cm.S for cm in calls)
    nc = _build_program(prm, calls, mms_by_sw, icols, scols)
    return nc, inputs


def kernel(x, edge_index, W):
    prm = Prm(N=int(x.shape[0]))
    nc, inputs = _get_program_and_prep(x, edge_index, W, prm)
    res = run_bass_kernel_spmd(nc, inputs, list(range(prm.C)))
    y = np.concatenate([res.results[c]["y"] for c in range(prm.C)], axis=0)
    return y.astype(np.float32)


def run_with_trace(x, edge_index, W, trace_cores=None):
    """test.py helper: returns (y, BassKernelResults) with profiling."""
    prm = Prm(N=int(x.shape[0]))
    nc, inputs = _get_program_and_prep(x, edge_index, W, prm)
    res = run_bass_kernel_spmd(
        nc, inputs, list(range(prm.C)), trace=True, trace_cores=trace_cores
    )
    y = np.concatenate([res.results[c]["y"] for c in range(prm.C)], axis=0)
    return y.astype(np.float32), res


# revision 66
# speedup vs baseline: 1.1964x; 1.1964x over previous
"""GCN inference kernel (y = D^-1/2 A D^-1/2 (x @ W.T)) on 8 Trainium2 NeuronCores.

Strategy (full inputs in, full output out; sharded internally):
  - Destination nodes are sharded across the 8 cores (12500 dsts each);
    edges are owned by the core that owns their dst, so the segment-sum is
    core-local (per the sharding hint).
  - Phase A (sharded): each core computes the scaled projection table
    h~[n] = dinv[n] * (x[n] @ W.T) for its 12800-node shard with PE matmuls
    (bf16), writing bf16 rows padded to 256B (SWDGE gather elem_size must
    be a multiple of 256B). An AllGather assembles the full table in DRAM.
  - Phase B (per core): ONE SWDGE dma_gather per (superwindow, bucket)
    streams h~[src] rows for the core's dst-sorted edge list into SBUF
    (~4.5k descriptors per call — the per-call Pool-engine desc-gen
    overhead was the baseline bottleneck at 392 small calls); a one-hot
    selection matrix B (one DVE is_equal per call, bf16, dst-local ids vs
    an iota row) turns the segment-sum into PE matmuls accumulated in PSUM
    per 128-dst tile; a final per-dst dinv scale lands y.
  - Per-core edges pack densely per call (trailing idx=-1 slots are
    skipped via a runtime descriptor count loaded into a ROTATING pool of
    8 GPSIMD registers -- a single register serializes desc-gen behind the
    previous call's DMA completion). A 48KB descriptor carveout gives the
    SWDGE rings a 3072-descriptor pipeline window.
  - All data-dependent structure (edge sort, padding, gather indices,
    one-hot ids, uniform per-core slice schedule) is prepared host-side in
    numpy; the device program is identical on all 8 cores (SPMD), only the
    per-core input arrays differ.
"""

import contextlib
import math
from dataclasses import dataclass, field

import ml_dtypes
import numpy as np

import concourse.bacc as bacc
import concourse.bass as bass
import concourse.mybir as mybir
import concourse.tile as tile
from concourse import library_config
from concourse.bass_utils import run_bass_kernel_spmd

P = 128  # SBUF partitions
FIN = 128
FOUT = 64
TROW = FOUT  # PACKED table row: 64 bf16 = 128B (no pad)
GROW = 2 * FOUT  # gather element: a PAIR of packed rows = 256B


@dataclass
class Prm:
    N: int = 100000  # nodes
    C: int = 8  # cores
    WG: int = 640  # nodes per phase-A write group
    GQ: int = 5  # write groups per quarter (= gather bucket)
    SWD: int = 512  # dst nodes per superwindow (TPSW * P)
    GBUFS: int = 6  # gather/one-hot pool depth (in calls)
    J: int = field(init=False)
    NS: int = field(init=False)  # dst shard size per core
    N2: int = field(init=False)  # padded node count (multiple of C*WG)
    NG: int = field(init=False)  # phase-A write groups
    NGpc: int = field(init=False)  # phase-A write groups per core
    NBK: int = field(init=False)  # gather buckets (int16 idx limit)
    SHN: int = field(init=False)  # nodes per phase-A shard
    QN: int = field(init=False)  # nodes per (core, quarter)
    BKCAP: int = field(init=False)  # table rows per gather bucket
    TBLR: int = field(init=False)  # total table rows
    TPSW: int = field(init=False)  # dst tiles per superwindow
    NSW: int = field(init=False)  # superwindows per core

    def __post_init__(self):
        assert self.WG % P == 0
        assert self.SWD % P == 0
        assert self.N % self.C == 0
        self.J = self.WG // P
        self.NS = self.N // self.C
        blk = self.C * self.WG
        self.N2 = ((self.N + blk - 1) // blk) * blk
        self.NG = self.N2 // self.WG
        self.NGpc = self.NG // self.C
        assert self.NGpc % self.GQ == 0
        self.NBK = self.NGpc // self.GQ
        self.SHN = self.N2 // self.C
        self.QN = self.GQ * self.WG
        self.BKCAP = self.C * self.QN
        assert self.BKCAP <= 32767
        self.TBLR = self.N2
        self.TPSW = self.SWD // P
        self.NSW = (self.NS + self.SWD - 1) // self.SWD


def _rmap(prm, n):
    """node id -> table row, quarter-major layout matching the single
    AllGather's concatenation of per-core shards (4 small per-quarter
    collectives measured SLOWER: ~25us fixed overhead each, serialized on
    the CC cores, so the last bucket landed at 247us vs 176us)."""
    c = n // prm.SHN
    i2 = n % prm.SHN
    k = i2 // prm.QN
    i = i2 % prm.QN
    wrap = prm.WG * (i // prm.WG) + prm.J * (i % P) + (i % prm.WG) // P
    return k * prm.BKCAP + c * prm.QN + wrap


def _wrap_idx(vals16):
    """[K] int16 (K % 128 == 0) -> [128, K//16] wrapped+replicated layout."""
    k = vals16.shape[0]
    w16 = vals16.reshape(k // 16, 16).T  # [16, K/16]
    return np.tile(w16, (8, 1))  # [128, K/16]


@dataclass
class CallMeta:
    sw: int
    bk: int
    S: int  # gather slices in this call (one dma_gather per call)
    SB: int  # one-hot B columns (>= S: boundary slices get per-tile masks)
    icol: int  # column offset into gidx array (8 * slice offset)
    scol: int  # column offset into dstl array (B-column offset)
    bslices: list  # [(sl, t)] B-column schedule, index = local B column


def _host_prep(x, edge_index, W, prm):
    N, C, NS = prm.N, prm.C, prm.NS
    src = np.asarray(edge_index[0], dtype=np.int64).astype(np.int32)
    dst = np.asarray(edge_index[1], dtype=np.int64).astype(np.int32)
    x = np.asarray(x, dtype=np.float32)
    W = np.asarray(W, dtype=np.float32)

    deg = np.bincount(dst, minlength=N).astype(np.float64)
    dinv = np.where(deg > 0, 1.0 / np.sqrt(np.maximum(deg, 1.0)), 0.0).astype(
        np.float32
    )

    # gather-order node map
    r_of = _rmap(prm, np.arange(N, dtype=np.int64)).astype(np.int64)
    bk_of = (r_of // prm.BKCAP).astype(np.int32)
    rel_of = (r_of % prm.BKCAP).astype(np.int16)

    # per-edge attributes
    core_e = dst // NS
    edl = dst - core_e * NS
    sw_e = edl // prm.SWD
    t_e = (edl % prm.SWD) // P
    q_e = (edl % P).astype(np.float32)
    bk_e = bk_of[src]
    rel_e = rel_of[src]
    par_e = (rel_e % 2).astype(np.int32)  # which half of the 256B pair
    pair_e = (rel_e // 2).astype(np.int16)  # gather element index
    tp_e = t_e * 2 + par_e  # purity class: (dst tile, parity)

    # per-core call structure: one call per (sw, bk); within a call the
    # core's REAL edges are packed densely (sorted by dst tile, then table
    # row for HBM locality), trailing slots hold idx=-1 and are skipped by
    # the runtime descriptor count (num_idxs_reg) -- no padding packets.
    ncalls = prm.NSW * prm.NBK
    counts = np.zeros((C, ncalls), dtype=np.int64)
    percore = []
    for c in range(C):
        m = core_e == c
        order = np.lexsort((pair_e[m], tp_e[m], bk_e[m], sw_e[m]))
        call = sw_e[m] * prm.NBK + bk_e[m]
        counts[c] = np.bincount(call, minlength=ncalls)
        percore.append(
            {
                "rel": pair_e[m][order],
                "q": q_e[m][order],
                "t": tp_e[m][order],  # purity class (t*2 + parity)
                "call": call[order],
            }
        )

    def build_sched(counts, percore, ncalls, cl_to_swbk, ensure):
        """Uniform union schedule + per-core data arrays: each core's REAL
        edges pack densely (sorted by tile then table row); trailing slots
        hold idx=-1 and are skipped by num_idxs_reg. B columns: union over
        cores of (slice, tile) incidences (boundary slices straddling two
        tiles get one masked B column per tile).

        Returns (calls, mms_by_sw, gidx_all, dstl_all, nval_all)."""
        S_call = ((counts.max(axis=0) + P - 1) // P).clip(min=1)
        ranks = []
        tile_sets = [dict() for _ in range(ncalls)]  # call -> {(sl,t)}
        for c in range(C):
            pc = percore[c]
            ne = pc["call"].shape[0]
            starts = np.zeros(ncalls, dtype=np.int64)
            np.cumsum(counts[c][:-1], out=starts[1:])
            rank = np.arange(ne, dtype=np.int64) - starts[pc["call"]]
            ranks.append(rank)
            sl = rank // P
            for cl, s, t in zip(
                *np.unique(
                    np.stack([pc["call"], sl, pc["t"]]), axis=1
                ).tolist()  # unique (call, sl, t) triples
            ):
                tile_sets[cl][(s, t)] = True
        if ensure:
            # every in-range (sw, t) opens its PSUM group somewhere
            for sw in range(prm.NSW):
                ntile = min(prm.TPSW, max(0, -(-(NS - sw * prm.SWD) // P)))
                cls = [cl for cl in range(ncalls) if cl_to_swbk(cl)[0] == sw]
                have = {tp // 2 for cl in cls for _, tp in tile_sets[cl]}
                for t in range(ntile):
                    if t not in have:
                        tile_sets[cls[0]][(0, 2 * t)] = True

        calls = []
        icol = 0
        scol = 0
        for cl in range(ncalls):
            sw, bk = cl_to_swbk(cl)
            S = int(S_call[cl])
            bslices = sorted(tile_sets[cl].keys())
            SB = len(bslices)
            calls.append(CallMeta(sw, bk, S, SB, icol, scol, bslices))
            icol += 8 * S
            scol += SB
        icols, scols = icol, scol

        # matmul schedule: call-major per sw; each (sw, t) accumulation
        # group opens on its first B-column and closes on its last
        by_sw = {}
        for ci, cm in enumerate(calls):
            by_sw.setdefault(cm.sw, []).append(ci)
        mms_by_sw = []
        for sw in range(prm.NSW):
            tot = [0] * prm.TPSW
            for ci in by_sw.get(sw, []):
                for _, tp in calls[ci].bslices:
                    tot[tp // 2] += 1
            seen = [0] * prm.TPSW
            mms = []
            for ci in by_sw.get(sw, []):
                cm = calls[ci]
                for bcol, (sl, tp) in enumerate(cm.bslices):
                    t, par = tp // 2, tp % 2
                    mms.append(
                        (ci, t, sl, bcol, par,
                         seen[t] == 0, seen[t] == tot[t] - 1)
                    )
                    seen[t] += 1
            mms_by_sw.append(mms)

        gidx_all = np.full((C, P, icols), -1, dtype=np.int16)
        dstl_all = np.full((C, P, scols), -1.0, dtype=ml_dtypes.bfloat16)
        nval_all = np.zeros((C, ncalls), dtype=np.int32)
        bmap = {}
        for ci, cm in enumerate(calls):
            for bcol, (sl, t) in enumerate(cm.bslices):
                bmap[(ci, sl, t)] = cm.scol + bcol
        for c in range(C):
            pc = percore[c]
            rank = ranks[c]
            ecol = np.array(
                [
                    bmap[(cl, s, t)]
                    for cl, s, t in zip(
                        pc["call"].tolist(),
                        (rank // P).tolist(),
                        pc["t"].tolist(),
                    )
                ],
                dtype=np.int64,
            ) if rank.shape[0] else np.zeros(0, dtype=np.int64)
            dstl_all[c, (rank % P), ecol] = pc["q"]
            cum = np.concatenate(([0], np.cumsum(counts[c])))
            for ci, cm in enumerate(calls):
                nv = int(counts[c][ci])
                seg = np.full(cm.S * P, -1, dtype=np.int16)
                seg[:nv] = pc["rel"][cum[ci] : cum[ci] + nv]
                if nv == 0:
                    # the gather ucode (and sim) need >= 1 valid index
                    seg[0] = 0
                    nv = 1
                nval_all[c, ci] = nv
                gidx_all[c, :, cm.icol : cm.icol + 8 * cm.S] = _wrap_idx(seg)
        return calls, mms_by_sw, gidx_all, dstl_all, nval_all

    calls, mms_by_sw, gidx_all, dstl_all, nval_all = build_sched(
        counts, percore, ncalls, lambda cl: (cl // prm.NBK, cl % prm.NBK), True
    )

    # phase-A inputs
    xT = np.zeros((FIN, prm.N2), dtype=ml_dtypes.bfloat16)
    xT[:, :N] = x.T.astype(ml_dtypes.bfloat16)
    WT = np.ascontiguousarray(W.T).astype(ml_dtypes.bfloat16)  # [FIN, FOUT]
    dpad = np.zeros(prm.N2, dtype=np.float32)
    dpad[:N] = dinv
    iota = np.broadcast_to(
        np.arange(P, dtype=ml_dtypes.bfloat16)[None, :], (P, P)
    ).copy()
    dinvD = np.zeros((C, P, prm.NSW * prm.TPSW), dtype=np.float32)
    w_idx = np.arange(prm.NSW * prm.TPSW)
    for c in range(C):
        node = c * NS + w_idx[:, None] * P + np.arange(P)[None, :]
        ok = node < (c + 1) * NS
        dv = np.where(ok, dinv[np.minimum(node, N - 1)], 0.0)
        dinvD[c][np.arange(P)[None, :], w_idx[:, None]] = dv

    # phase-A shard for core c: the nodes whose table rows fall in its
    # AllGather output block [SHN*c, SHN*(c+1)) of the quarter-major layout:
    # quarter (c//2) of original node shards 4*(c%2) .. 4*(c%2)+3
    inputs = []
    i2 = np.arange(prm.SHN)
    for c in range(C):
        segs = [
            np.arange(prm.QN, dtype=np.int64)
            + (4 * (c % 2) + u) * prm.SHN
            + (c // 2) * prm.QN
            for u in range(C // 2)
        ]
        nodes = np.concatenate(segs)  # SHN nodes in TBSH write order
        assert nodes.shape[0] == prm.SHN
        seq = dpad[nodes]
        dinvA_c = np.zeros((P, prm.NGpc * prm.J), dtype=np.float32)
        dinvA_c[i2 % P, (i2 // prm.WG) * prm.J + (i2 % prm.WG) // P] = seq
        inputs.append(
            {
                "xT": np.ascontiguousarray(xT[:, nodes]),
                "WT": WT,
                "dinvA": dinvA_c,
                "iota": iota,
                "dinvD": dinvD[c],
                "gidx": gidx_all[c],
                "dstl": dstl_all[c],
                "nval": nval_all[c : c + 1],
            }
        )
    return inputs, calls, mms_by_sw


def _split_sync_waits(nc):
    """This env's walrus rejects >1 sync wait on some opcodes; keep 1 wait
    per instruction, moving extras onto preceding same-engine NOPs."""
    for bb in nc.main_func.blocks:
        insts = bb.instructions
        i = 0
        while i < len(insts):
            ins = insts[i]
            si = ins.sync_info
            if si is not None and si.on_wait is not None and len(si.on_wait) > 1:
                waits = list(si.on_wait)
                keep, extra = waits[-1:], waits[:-1]
                k = 0
                while extra:
                    chunk, extra = extra[:1], extra[1:]
                    nop = mybir.InstNoOp(name=f"{ins.name}-ws{k}", ins=[], outs=[])
                    nop.engine = ins.engine
                    nop.sync_info = mybir.SyncInfo(on_wait=chunk, on_update=[])
                    nc.register_instruction(nop)
                    insts.insert(i, nop)
                    i += 1
                    k += 1
                ins.sync_info = mybir.SyncInfo(
                    on_wait=keep, on_update=list(si.on_update or [])
                )
            i += 1


def _build_program(prm, calls, mms_by_sw):
    f32 = mybir.dt.float32
    bf16 = mybir.dt.bfloat16
    icols = sum(8 * cm.S for cm in calls)
    scols = sum(cm.SB for cm in calls)
    # 48KB/partition descriptor carveout -> 3072-desc SWDGE ring per queue
    # (default 16KB/1024 descs stalls desc-gen at transfer pace with almost
    # no pipeline buffer; the ring frees as transfers COMPLETE)
    nc = bacc.Bacc(
        "TRN2", num_swdge_queues=4, dynamic_dma_scratch_size=40960
    )

    NGpc = prm.NGpc
    xT = nc.declare_dram_parameter(
        "xT", [FIN, NGpc * prm.WG], bf16, isOutput=False
    )
    WT = nc.declare_dram_parameter("WT", [FIN, FOUT], bf16, isOutput=False)
    dinvA = nc.declare_dram_parameter(
        "dinvA", [P, NGpc * prm.J], f32, isOutput=False
    )
    iota = nc.declare_dram_parameter("iota", [P, P], bf16, isOutput=False)
    dinvD = nc.declare_dram_parameter(
        "dinvD", [P, prm.NSW * prm.TPSW], f32, isOutput=False
    )
    gidx = nc.declare_dram_parameter("gidx", [P, icols], mybir.dt.int16, isOutput=False)
    dstl = nc.declare_dram_parameter("dstl", [P, scols], bf16, isOutput=False)
    nval = nc.declare_dram_parameter(
        "nval", [1, len(calls)], mybir.dt.int32, isOutput=False
    )
    y = nc.declare_dram_parameter("y", [prm.NS, FOUT], f32, isOutput=True)
    # phase A is SHARDED: each core computes its 12800-row table shard, an
    # AllGather assembles the full table.
    TBSH = nc.dram_tensor("tbsh", [NGpc * prm.WG, TROW], bf16)  # packed 128B rows
    TBLA = nc.dram_tensor(
        "tbla", [prm.TBLR // 2, GROW], bf16, addr_space="Shared"
    )  # viewed as 256B row-PAIR elements for the gather

    S_MAX = max(cm.S for cm in calls)
    SB_MAX = max(cm.SB for cm in calls)
    calls_by_swbk = {}
    for ci, cm in enumerate(calls):
        calls_by_swbk[(cm.sw, cm.bk)] = (ci, cm)

    # gidx/dstl are loaded in CHUNK_SW-superwindow mega-chunks (few large
    # HWDGE descriptors instead of one small load per call)
    CHUNK_SW = 5
    NCH = (prm.NSW + CHUNK_SW - 1) // CHUNK_SW
    ch_i0 = []  # (icol0, icol1, scol0, scol1) per chunk
    for ch in range(NCH):
        sws = [cm for cm in calls if ch * CHUNK_SW <= cm.sw < (ch + 1) * CHUNK_SW]
        i0 = min(cm.icol for cm in sws)
        i1 = max(cm.icol + 8 * cm.S for cm in sws)
        s0 = min(cm.scol for cm in sws)
        s1 = max(cm.scol + cm.SB for cm in sws)
        ch_i0.append((i0, i1, s0, s1))
    ICH_MAX = max(i1 - i0 for i0, i1, _, _ in ch_i0)
    SCH_MAX = max(s1 - s0 for _, _, s0, s1 in ch_i0)

    with tile.TileContext(nc) as tc:
        _stk = contextlib.ExitStack()
        cpool = _stk.enter_context(tc.tile_pool(name="const", bufs=1))
        pa = _stk.enter_context(tc.tile_pool(name="pa", bufs=3))
        psa = _stk.enter_context(tc.tile_pool(name="psa", bufs=2, space="PSUM"))
        pidx = _stk.enter_context(tc.tile_pool(name="pidx", bufs=2))
        pg = _stk.enter_context(tc.tile_pool(name="pg", bufs=prm.GBUFS))
        pb = _stk.enter_context(tc.tile_pool(name="pb", bufs=prm.GBUFS))
        py = _stk.enter_context(tc.tile_pool(name="py", bufs=3))
        psb = _stk.enter_context(tc.tile_pool(name="psb", bufs=1, space="PSUM"))

        wt_sb = cpool.tile([FIN, FOUT], bf16, tag="wt")
        nc.sync.dma_start(out=wt_sb[:], in_=WT[:])
        dinvA_sb = cpool.tile([P, NGpc * prm.J], f32, tag="da")
        nc.sync.dma_start(out=dinvA_sb[:], in_=dinvA[:])
        iota_sb = cpool.tile([P, P], bf16, tag="io")
        nc.sync.dma_start(out=iota_sb[:], in_=iota[:])
        dinvD_sb = cpool.tile([P, prm.NSW * prm.TPSW], f32, tag="dd")
        nc.sync.dma_start(out=dinvD_sb[:], in_=dinvD[:])
        # materialized iota along the dst dim, constant along the B-column
        # dim: lets the one-hot is_equal keep ALL operands' inner AP dim
        # packed (stride 1), which enables the DVE 2x_1p perf mode (the
        # broadcast-inner layout runs at half rate)
        iotaF_sb = cpool.tile([P, P, SB_MAX], bf16, tag="iof")
        nc.vector.tensor_tensor(
            out=iotaF_sb[:],
            in0=iota_sb[:, :, None].to_broadcast([P, P, SB_MAX]),
            in1=iota_sb[:, :, None].to_broadcast([P, P, SB_MAX]),
            op=mybir.AluOpType.max,
        )
        nval_sb = cpool.tile([1, len(calls)], mybir.dt.int32, tag="nv")
        nc.sync.dma_start(out=nval_sb[:], in_=nval[:])
        # rotating register pool: a single count register serializes call
        # k+1's reg_load behind call k's FULL DMA completion (register-WAR
        # tracking) — with 8, the dependency lands 8 calls back
        nregs = [nc.gpsimd.alloc_register(f"nvreg{i}") for i in range(8)]
        emitted = [0]
        rctr = [0]

        chunk_tiles = {}

        def load_chunk(ch):
            if ch >= NCH or ch in chunk_tiles:
                return
            idx_t = pidx.tile([P, ICH_MAX], mybir.dt.int16, tag="idx")
            dst_t = pidx.tile([P, SCH_MAX], bf16, tag="dst")
            i0, i1, s0, s1 = ch_i0[ch]
            nc.sync.dma_start(out=idx_t[:, : i1 - i0], in_=gidx[:, i0:i1])
            nc.sync.dma_start(out=dst_t[:, : s1 - s0], in_=dstl[:, s0:s1])
            chunk_tiles[ch] = (idx_t, dst_t)

        load_chunk(0)

        # ------- Phase A: build the table shard; per-quarter AllGather ----
        # collective q fires as soon as the 5 write groups of quarter q are
        # in TBSH, so bucket-q gathers pipeline with the rest of phase A
        for g in range(NGpc):
            xt = pa.tile([P, prm.WG], bf16, tag="xt")
            nc.sync.dma_start(
                out=xt[:], in_=xT[:, g * prm.WG : (g + 1) * prm.WG]
            )
            hps = psa.tile([P, prm.J * FOUT], f32, tag="hps")
            for j in range(prm.J):
                nc.tensor.matmul(
                    out=hps[:, j * FOUT : (j + 1) * FOUT],
                    lhsT=xt[:, j * P : (j + 1) * P],
                    rhs=wt_sb[:],
                    start=True,
                    stop=True,
                )
            tsb = pa.tile([P, prm.J, TROW], bf16, tag="tsb")
            nc.vector.tensor_tensor(
                out=tsb[:, :, :FOUT],
                in0=hps[:].rearrange("p (j f) -> p j f", f=FOUT),
                in1=dinvA_sb[:, g * prm.J : (g + 1) * prm.J][
                    :, :, None
                ].to_broadcast([P, prm.J, FOUT]),
                op=mybir.AluOpType.mult,
            )
            base = prm.WG * g
            nc.sync.dma_start(
                out=TBSH[base : base + prm.WG, :].rearrange(
                    "(p j) f -> p j f", j=prm.J
                ),
                in_=tsb[:],
            )
        # assemble the full table from all cores' shards
        nc.gpsimd.collective_compute(
            "AllGather",
            mybir.AluOpType.bypass,
            replica_groups=[list(range(prm.C))],
            ins=[TBSH[:]],
            outs=[TBLA[:]],
        )
        TBL = [
            TBLA[k * (prm.BKCAP // 2) : (k + 1) * (prm.BKCAP // 2), :]
            for k in range(prm.NBK)
        ]

        # ---------------- Phase B/L: gather + segment-sum ----------------
        def emit_call(ci, cm, tiles):
            S, SB = cm.S, cm.SB
            ch = cm.sw // CHUNK_SW
            idx_t, dst_t = chunk_tiles[ch]
            io = cm.icol - ch_i0[ch][0]
            so = cm.scol - ch_i0[ch][2]
            g_t = pg.tile([P, S_MAX, GROW], bf16, tag="g")
            # slots skipped by the runtime count must hold finite data
            # (0 * NaN would poison the PE accumulation): zero each pool
            # buffer on its first use; afterwards stale rows are old table
            # data, which is finite
            if emitted[0] < prm.GBUFS:
                nc.vector.memset(g_t[:], 0.0)
            emitted[0] += 1
            nreg = nregs[rctr[0] % len(nregs)]
            rctr[0] += 1
            nc.gpsimd.reg_load(nreg, nval_sb[0:1, ci : ci + 1])
            nc.gpsimd.dma_gather(
                out_ap=g_t[:, :S, :],
                in_ap=TBL[cm.bk],
                idxs_ap=idx_t[:, io : io + 8 * S],
                num_idxs=S * P,
                num_idxs_reg=nreg,
                elem_size=GROW,
                single_packet=False,
                queue_num=cm.bk % 4,
            )
            # transposed one-hot [slot, dst, bcol]: inner dim (bcol) packed
            # on every operand -> DVE 2x_1p
            b_t = pb.tile([P, P, SB_MAX], bf16, tag="b")
            nc.vector.tensor_tensor(
                out=b_t[:, :, :SB],
                in0=dst_t[:, so : so + SB][:, None, :].to_broadcast([P, P, SB]),
                in1=iotaF_sb[:, :, :SB],
                op=mybir.AluOpType.is_equal,
            )
            tiles[ci] = (g_t, b_t)

        def emit_mms(sw, tiles, mms_src, yout, acc_pfx=""):
            # accumulation groups must never share a PSUM bank
            # (start=True clears the whole bank) - one [P, FOUT] tile per
            # dst tile, all TPSW open concurrently in separate banks
            rows_sw = min(prm.SWD, prm.NS - sw * prm.SWD)
            nt = (rows_sw + P - 1) // P  # valid dst tiles this sw
            ysc = py.tile([P, prm.TPSW, FOUT], f32, tag="ysc")
            mms = mms_src[sw]
            psum_t = {
                t: psb.tile([P, FOUT], f32, tag=f"acc{t}", name=f"{acc_pfx}acc{t}")
                for t in range(prm.TPSW)
            }
            closed = set()
            for ci, t, sl, bcol, par, st, sp in mms:
                g_t, b_t = tiles[ci]
                nc.tensor.matmul(
                    out=psum_t[t][:],
                    lhsT=b_t[:, :, bcol],
                    rhs=g_t[:, sl, par * FOUT : (par + 1) * FOUT],
                    start=st,
                    stop=sp,
                )
                if sp and t < nt:
                    # scale by dinv[dst] right after the group closes
                    w = sw * prm.TPSW + t
                    nc.scalar.activation(
                        out=ysc[:, t, :],
                        in_=psum_t[t][:],
                        func=mybir.ActivationFunctionType.Copy,
                        scale=dinvD_sb[:, w : w + 1],
                    )
                    closed.add(t)
            assert closed == set(range(nt)), (sw, closed, nt)
            for t in range(nt):
                rt = min(P, rows_sw - t * P)
                r0 = sw * prm.SWD + t * P
                nc.scalar.dma_start(
                    out=yout[r0 : r0 + rt, :], in_=ysc[:rt, t, :]
                )

        for sw in range(prm.NSW):
            if sw % CHUNK_SW == 0:
                load_chunk(sw // CHUNK_SW + 1)  # prefetch next chunk
            tiles = {}
            for bk in range(prm.NBK):
                cicm = calls_by_swbk.get((sw, bk))
                if cicm is not None:
                    emit_call(cicm[0], cicm[1], tiles)
            emit_mms(sw, tiles, mms_by_sw, y)
        _stk.close()

    nc.compile()
    _split_sync_waits(nc)
    return nc


def _get_program_and_prep(x, edge_index, W, prm):
    inputs, calls, mms_by_sw = _host_prep(x, edge_index, W, prm)
    nc = _build_program(prm, calls, mms_by_sw)
    return nc, inputs


def kernel(x, edge_index, W):
    prm = Prm(N=int(x.shape[0]))
    nc, inputs = _get_program_and_prep(x, edge_index, W, prm)
    res = run_bass_kernel_spmd(nc, inputs, list(range(prm.C)))
    y = np.concatenate([res.results[c]["y"] for c in range(prm.C)], axis=0)
    return y.astype(np.float32)


def run_with_trace(x, edge_index, W, trace_cores=None):
    """test.py helper: returns (y, BassKernelResults) with profiling."""
    prm = Prm(N=int(x.shape[0]))
    nc, inputs = _get_program_and_prep(x, edge_index, W, prm)
    res = run_bass_kernel_spmd(
        nc, inputs, list(range(prm.C)), trace=True, trace_cores=trace_cores
    )
    y = np.concatenate([res.results[c]["y"] for c in range(prm.C)], axis=0)
    return y.astype(np.float32), res


# revision 67
# speedup vs baseline: 1.2241x; 1.0231x over previous
"""GCN inference kernel (y = D^-1/2 A D^-1/2 (x @ W.T)) on 8 Trainium2 NeuronCores.

Strategy (full inputs in, full output out; sharded internally):
  - Destination nodes are sharded across the 8 cores (12500 dsts each);
    edges are owned by the core that owns their dst, so the segment-sum is
    core-local (per the sharding hint).
  - Phase A (sharded): each core computes the scaled projection table
    h~[n] = dinv[n] * (x[n] @ W.T) for its 12800-node shard with PE matmuls
    (bf16), writing bf16 rows padded to 256B (SWDGE gather elem_size must
    be a multiple of 256B). An AllGather assembles the full table in DRAM.
  - Phase B (per core): ONE SWDGE dma_gather per (superwindow, bucket)
    streams h~[src] rows for the core's dst-sorted edge list into SBUF
    (~4.5k descriptors per call — the per-call Pool-engine desc-gen
    overhead was the baseline bottleneck at 392 small calls); a one-hot
    selection matrix B (one DVE is_equal per call, bf16, dst-local ids vs
    an iota row) turns the segment-sum into PE matmuls accumulated in PSUM
    per 128-dst tile; a final per-dst dinv scale lands y.
  - Per-core edges pack densely per call (trailing idx=-1 slots are
    skipped via a runtime descriptor count loaded into a ROTATING pool of
    8 GPSIMD registers -- a single register serializes desc-gen behind the
    previous call's DMA completion). A 48KB descriptor carveout gives the
    SWDGE rings a 3072-descriptor pipeline window.
  - All data-dependent structure (edge sort, padding, gather indices,
    one-hot ids, uniform per-core slice schedule) is prepared host-side in
    numpy; the device program is identical on all 8 cores (SPMD), only the
    per-core input arrays differ.
"""

import contextlib
import math
from dataclasses import dataclass, field

import ml_dtypes
import numpy as np

import concourse.bacc as bacc
import concourse.bass as bass
import concourse.mybir as mybir
import concourse.tile as tile
from concourse import library_config
from concourse.bass_utils import run_bass_kernel_spmd

P = 128  # SBUF partitions
FIN = 128
FOUT = 64
TROW = FOUT  # PACKED table row: 64 bf16 = 128B (no pad)
GROW = 2 * FOUT  # gather element: a PAIR of packed rows = 256B


@dataclass
class Prm:
    N: int = 100000  # nodes
    C: int = 8  # cores
    WG: int = 640  # nodes per phase-A write group
    GQ: int = 5  # write groups per quarter (= gather bucket)
    SWD: int = 384  # dst nodes per superwindow (TPSW * P)
    GBUFS: int = 6  # gather/one-hot pool depth (in calls)
    J: int = field(init=False)
    NS: int = field(init=False)  # dst shard size per core
    N2: int = field(init=False)  # padded node count (multiple of C*WG)
    NG: int = field(init=False)  # phase-A write groups
    NGpc: int = field(init=False)  # phase-A write groups per core
    NBK: int = field(init=False)  # gather buckets (int16 idx limit)
    SHN: int = field(init=False)  # nodes per phase-A shard
    QN: int = field(init=False)  # nodes per (core, quarter)
    BKCAP: int = field(init=False)  # table rows per gather bucket
    TBLR: int = field(init=False)  # total table rows
    TPSW: int = field(init=False)  # dst tiles per superwindow
    NSW: int = field(init=False)  # superwindows per core

    def __post_init__(self):
        assert self.WG % P == 0
        assert self.SWD % P == 0
        assert self.N % self.C == 0
        self.J = self.WG // P
        self.NS = self.N // self.C
        blk = self.C * self.WG
        self.N2 = ((self.N + blk - 1) // blk) * blk
        self.NG = self.N2 // self.WG
        self.NGpc = self.NG // self.C
        assert self.NGpc % self.GQ == 0
        self.NBK = self.NGpc // self.GQ
        self.SHN = self.N2 // self.C
        self.QN = self.GQ * self.WG
        self.BKCAP = self.C * self.QN
        assert self.BKCAP <= 32767
        self.TBLR = self.N2
        self.TPSW = self.SWD // P
        self.NSW = (self.NS + self.SWD - 1) // self.SWD


def _rmap(prm, n):
    """node id -> table row, quarter-major layout matching the single
    AllGather's concatenation of per-core shards (4 small per-quarter
    collectives measured SLOWER: ~25us fixed overhead each, serialized on
    the CC cores, so the last bucket landed at 247us vs 176us)."""
    c = n // prm.SHN
    i2 = n % prm.SHN
    k = i2 // prm.QN
    i = i2 % prm.QN
    wrap = prm.WG * (i // prm.WG) + prm.J * (i % P) + (i % prm.WG) // P
    return k * prm.BKCAP + c * prm.QN + wrap


def _wrap_idx(vals16):
    """[K] int16 (K % 128 == 0) -> [128, K//16] wrapped+replicated layout."""
    k = vals16.shape[0]
    w16 = vals16.reshape(k // 16, 16).T  # [16, K/16]
    return np.tile(w16, (8, 1))  # [128, K/16]


@dataclass
class CallMeta:
    sw: int
    bk: int
    S: int  # gather slices in this call (one dma_gather per call)
    SB: int  # one-hot B columns (>= S: boundary slices get per-tile masks)
    icol: int  # column offset into gidx array (8 * slice offset)
    scol: int  # column offset into dstl array (B-column offset)
    bslices: list  # [(sl, t)] B-column schedule, index = local B column


def _host_prep(x, edge_index, W, prm):
    N, C, NS = prm.N, prm.C, prm.NS
    src = np.asarray(edge_index[0], dtype=np.int64).astype(np.int32)
    dst = np.asarray(edge_index[1], dtype=np.int64).astype(np.int32)
    x = np.asarray(x, dtype=np.float32)
    W = np.asarray(W, dtype=np.float32)

    deg = np.bincount(dst, minlength=N).astype(np.float64)
    dinv = np.where(deg > 0, 1.0 / np.sqrt(np.maximum(deg, 1.0)), 0.0).astype(
        np.float32
    )

    # gather-order node map
    r_of = _rmap(prm, np.arange(N, dtype=np.int64)).astype(np.int64)
    bk_of = (r_of // prm.BKCAP).astype(np.int32)
    rel_of = (r_of % prm.BKCAP).astype(np.int16)

    # per-edge attributes
    core_e = dst // NS
    edl = dst - core_e * NS
    sw_e = edl // prm.SWD
    t_e = (edl % prm.SWD) // P
    q_e = (edl % P).astype(np.float32)
    bk_e = bk_of[src]
    rel_e = rel_of[src]
    par_e = (rel_e % 2).astype(np.int32)  # which half of the 256B pair
    pair_e = (rel_e // 2).astype(np.int16)  # gather element index
    tp_e = t_e * 2 + par_e  # purity class: (dst tile, parity)

    # per-core call structure: one call per (sw, bk); within a call the
    # core's REAL edges are packed densely (sorted by dst tile, then table
    # row for HBM locality), trailing slots hold idx=-1 and are skipped by
    # the runtime descriptor count (num_idxs_reg) -- no padding packets.
    ncalls = prm.NSW * prm.NBK
    counts = np.zeros((C, ncalls), dtype=np.int64)
    percore = []
    for c in range(C):
        m = core_e == c
        order = np.lexsort((pair_e[m], tp_e[m], bk_e[m], sw_e[m]))
        call = sw_e[m] * prm.NBK + bk_e[m]
        counts[c] = np.bincount(call, minlength=ncalls)
        percore.append(
            {
                "rel": pair_e[m][order],
                "q": q_e[m][order],
                "t": tp_e[m][order],  # purity class (t*2 + parity)
                "call": call[order],
            }
        )

    def build_sched(counts, percore, ncalls, cl_to_swbk, ensure):
        """Uniform union schedule + per-core data arrays: each core's REAL
        edges pack densely (sorted by tile then table row); trailing slots
        hold idx=-1 and are skipped by num_idxs_reg. B columns: union over
        cores of (slice, tile) incidences (boundary slices straddling two
        tiles get one masked B column per tile).

        Returns (calls, mms_by_sw, gidx_all, dstl_all, nval_all)."""
        S_call = ((counts.max(axis=0) + P - 1) // P).clip(min=1)
        ranks = []
        tile_sets = [dict() for _ in range(ncalls)]  # call -> {(sl,t)}
        for c in range(C):
            pc = percore[c]
            ne = pc["call"].shape[0]
            starts = np.zeros(ncalls, dtype=np.int64)
            np.cumsum(counts[c][:-1], out=starts[1:])
            rank = np.arange(ne, dtype=np.int64) - starts[pc["call"]]
            ranks.append(rank)
            sl = rank // P
            for cl, s, t in zip(
                *np.unique(
                    np.stack([pc["call"], sl, pc["t"]]), axis=1
                ).tolist()  # unique (call, sl, t) triples
            ):
                tile_sets[cl][(s, t)] = True
        if ensure:
            # every in-range (sw, t) opens its PSUM group somewhere
            for sw in range(prm.NSW):
                ntile = min(prm.TPSW, max(0, -(-(NS - sw * prm.SWD) // P)))
                cls = [cl for cl in range(ncalls) if cl_to_swbk(cl)[0] == sw]
                have = {tp // 2 for cl in cls for _, tp in tile_sets[cl]}
                for t in range(ntile):
                    if t not in have:
                        tile_sets[cls[0]][(0, 2 * t)] = True

        calls = []
        icol = 0
        scol = 0
        for cl in range(ncalls):
            sw, bk = cl_to_swbk(cl)
            S = int(S_call[cl])
            bslices = sorted(tile_sets[cl].keys())
            SB = len(bslices)
            calls.append(CallMeta(sw, bk, S, SB, icol, scol, bslices))
            icol += 8 * S
            scol += SB
        icols, scols = icol, scol

        # matmul schedule: call-major per sw; each (sw, t) accumulation
        # group opens on its first B-column and closes on its last
        by_sw = {}
        for ci, cm in enumerate(calls):
            by_sw.setdefault(cm.sw, []).append(ci)
        mms_by_sw = []
        for sw in range(prm.NSW):
            tot = [0] * prm.TPSW
            for ci in by_sw.get(sw, []):
                for _, tp in calls[ci].bslices:
                    tot[tp // 2] += 1
            seen = [0] * prm.TPSW
            mms = []
            for ci in by_sw.get(sw, []):
                cm = calls[ci]
                for bcol, (sl, tp) in enumerate(cm.bslices):
                    t, par = tp // 2, tp % 2
                    mms.append(
                        (ci, t, sl, bcol, par,
                         seen[t] == 0, seen[t] == tot[t] - 1)
                    )
                    seen[t] += 1
            mms_by_sw.append(mms)

        gidx_all = np.full((C, P, icols), -1, dtype=np.int16)
        dstl_all = np.full((C, P, scols), -1.0, dtype=ml_dtypes.bfloat16)
        nval_all = np.zeros((C, ncalls), dtype=np.int32)
        bmap = {}
        for ci, cm in enumerate(calls):
            for bcol, (sl, t) in enumerate(cm.bslices):
                bmap[(ci, sl, t)] = cm.scol + bcol
        for c in range(C):
            pc = percore[c]
            rank = ranks[c]
            ecol = np.array(
                [
                    bmap[(cl, s, t)]
                    for cl, s, t in zip(
                        pc["call"].tolist(),
                        (rank // P).tolist(),
                        pc["t"].tolist(),
                    )
                ],
                dtype=np.int64,
            ) if rank.shape[0] else np.zeros(0, dtype=np.int64)
            dstl_all[c, (rank % P), ecol] = pc["q"]
            cum = np.concatenate(([0], np.cumsum(counts[c])))
            for ci, cm in enumerate(calls):
                nv = int(counts[c][ci])
                seg = np.full(cm.S * P, -1, dtype=np.int16)
                seg[:nv] = pc["rel"][cum[ci] : cum[ci] + nv]
                if nv == 0:
                    # the gather ucode (and sim) need >= 1 valid index
                    seg[0] = 0
                    nv = 1
                nval_all[c, ci] = nv
                gidx_all[c, :, cm.icol : cm.icol + 8 * cm.S] = _wrap_idx(seg)
        return calls, mms_by_sw, gidx_all, dstl_all, nval_all

    calls, mms_by_sw, gidx_all, dstl_all, nval_all = build_sched(
        counts, percore, ncalls, lambda cl: (cl // prm.NBK, cl % prm.NBK), True
    )

    # phase-A inputs
    xT = np.zeros((FIN, prm.N2), dtype=ml_dtypes.bfloat16)
    xT[:, :N] = x.T.astype(ml_dtypes.bfloat16)
    WT = np.ascontiguousarray(W.T).astype(ml_dtypes.bfloat16)  # [FIN, FOUT]
    dpad = np.zeros(prm.N2, dtype=np.float32)
    dpad[:N] = dinv
    iota = np.broadcast_to(
        np.arange(P, dtype=ml_dtypes.bfloat16)[None, :], (P, P)
    ).copy()
    dinvD = np.zeros((C, P, prm.NSW * prm.TPSW), dtype=np.float32)
    w_idx = np.arange(prm.NSW * prm.TPSW)
    for c in range(C):
        node = c * NS + w_idx[:, None] * P + np.arange(P)[None, :]
        ok = node < (c + 1) * NS
        dv = np.where(ok, dinv[np.minimum(node, N - 1)], 0.0)
        dinvD[c][np.arange(P)[None, :], w_idx[:, None]] = dv

    # phase-A shard for core c: the nodes whose table rows fall in its
    # AllGather output block [SHN*c, SHN*(c+1)) of the quarter-major layout:
    # quarter (c//2) of original node shards 4*(c%2) .. 4*(c%2)+3
    inputs = []
    i2 = np.arange(prm.SHN)
    for c in range(C):
        segs = [
            np.arange(prm.QN, dtype=np.int64)
            + (4 * (c % 2) + u) * prm.SHN
            + (c // 2) * prm.QN
            for u in range(C // 2)
        ]
        nodes = np.concatenate(segs)  # SHN nodes in TBSH write order
        assert nodes.shape[0] == prm.SHN
        seq = dpad[nodes]
        dinvA_c = np.zeros((P, prm.NGpc * prm.J), dtype=np.float32)
        dinvA_c[i2 % P, (i2 // prm.WG) * prm.J + (i2 % prm.WG) // P] = seq
        inputs.append(
            {
                "xT": np.ascontiguousarray(xT[:, nodes]),
                "WT": WT,
                "dinvA": dinvA_c,
                "iota": iota,
                "dinvD": dinvD[c],
                "gidx": gidx_all[c],
                "dstl": dstl_all[c],
                "nval": nval_all[c : c + 1],
            }
        )
    return inputs, calls, mms_by_sw


def _split_sync_waits(nc):
    """This env's walrus rejects >1 sync wait on some opcodes; keep 1 wait
    per instruction, moving extras onto preceding same-engine NOPs."""
    for bb in nc.main_func.blocks:
        insts = bb.instructions
        i = 0
        while i < len(insts):
            ins = insts[i]
            si = ins.sync_info
            if si is not None and si.on_wait is not None and len(si.on_wait) > 1:
                waits = list(si.on_wait)
                keep, extra = waits[-1:], waits[:-1]
                k = 0
                while extra:
                    chunk, extra = extra[:1], extra[1:]
                    nop = mybir.InstNoOp(name=f"{ins.name}-ws{k}", ins=[], outs=[])
                    nop.engine = ins.engine
                    nop.sync_info = mybir.SyncInfo(on_wait=chunk, on_update=[])
                    nc.register_instruction(nop)
                    insts.insert(i, nop)
                    i += 1
                    k += 1
                ins.sync_info = mybir.SyncInfo(
                    on_wait=keep, on_update=list(si.on_update or [])
                )
            i += 1


def _build_program(prm, calls, mms_by_sw):
    f32 = mybir.dt.float32
    bf16 = mybir.dt.bfloat16
    icols = sum(8 * cm.S for cm in calls)
    scols = sum(cm.SB for cm in calls)
    # 48KB/partition descriptor carveout -> 3072-desc SWDGE ring per queue
    # (default 16KB/1024 descs stalls desc-gen at transfer pace with almost
    # no pipeline buffer; the ring frees as transfers COMPLETE)
    nc = bacc.Bacc(
        "TRN2", num_swdge_queues=4, dynamic_dma_scratch_size=49152
    )

    NGpc = prm.NGpc
    xT = nc.declare_dram_parameter(
        "xT", [FIN, NGpc * prm.WG], bf16, isOutput=False
    )
    WT = nc.declare_dram_parameter("WT", [FIN, FOUT], bf16, isOutput=False)
    dinvA = nc.declare_dram_parameter(
        "dinvA", [P, NGpc * prm.J], f32, isOutput=False
    )
    iota = nc.declare_dram_parameter("iota", [P, P], bf16, isOutput=False)
    dinvD = nc.declare_dram_parameter(
        "dinvD", [P, prm.NSW * prm.TPSW], f32, isOutput=False
    )
    gidx = nc.declare_dram_parameter("gidx", [P, icols], mybir.dt.int16, isOutput=False)
    dstl = nc.declare_dram_parameter("dstl", [P, scols], bf16, isOutput=False)
    nval = nc.declare_dram_parameter(
        "nval", [1, len(calls)], mybir.dt.int32, isOutput=False
    )
    y = nc.declare_dram_parameter("y", [prm.NS, FOUT], f32, isOutput=True)
    # phase A is SHARDED: each core computes its 12800-row table shard, an
    # AllGather assembles the full table.
    TBSH = nc.dram_tensor("tbsh", [NGpc * prm.WG, TROW], bf16)  # packed 128B rows
    TBLA = nc.dram_tensor(
        "tbla", [prm.TBLR // 2, GROW], bf16, addr_space="Shared"
    )  # viewed as 256B row-PAIR elements for the gather

    S_MAX = max(cm.S for cm in calls)
    SB_MAX = max(cm.SB for cm in calls)
    calls_by_swbk = {}
    for ci, cm in enumerate(calls):
        calls_by_swbk[(cm.sw, cm.bk)] = (ci, cm)

    # gidx/dstl are loaded in CHUNK_SW-superwindow mega-chunks (few large
    # HWDGE descriptors instead of one small load per call)
    CHUNK_SW = 5
    NCH = (prm.NSW + CHUNK_SW - 1) // CHUNK_SW
    ch_i0 = []  # (icol0, icol1, scol0, scol1) per chunk
    for ch in range(NCH):
        sws = [cm for cm in calls if ch * CHUNK_SW <= cm.sw < (ch + 1) * CHUNK_SW]
        i0 = min(cm.icol for cm in sws)
        i1 = max(cm.icol + 8 * cm.S for cm in sws)
        s0 = min(cm.scol for cm in sws)
        s1 = max(cm.scol + cm.SB for cm in sws)
        ch_i0.append((i0, i1, s0, s1))
    ICH_MAX = max(i1 - i0 for i0, i1, _, _ in ch_i0)
    SCH_MAX = max(s1 - s0 for _, _, s0, s1 in ch_i0)

    with tile.TileContext(nc) as tc:
        _stk = contextlib.ExitStack()
        cpool = _stk.enter_context(tc.tile_pool(name="const", bufs=1))
        pa = _stk.enter_context(tc.tile_pool(name="pa", bufs=3))
        psa = _stk.enter_context(tc.tile_pool(name="psa", bufs=2, space="PSUM"))
        pidx = _stk.enter_context(tc.tile_pool(name="pidx", bufs=2))
        pg = _stk.enter_context(tc.tile_pool(name="pg", bufs=prm.GBUFS))
        pb = _stk.enter_context(tc.tile_pool(name="pb", bufs=prm.GBUFS))
        py = _stk.enter_context(tc.tile_pool(name="py", bufs=3))
        psb = _stk.enter_context(tc.tile_pool(name="psb", bufs=1, space="PSUM"))

        wt_sb = cpool.tile([FIN, FOUT], bf16, tag="wt")
        nc.sync.dma_start(out=wt_sb[:], in_=WT[:])
        dinvA_sb = cpool.tile([P, NGpc * prm.J], f32, tag="da")
        nc.sync.dma_start(out=dinvA_sb[:], in_=dinvA[:])
        iota_sb = cpool.tile([P, P], bf16, tag="io")
        nc.sync.dma_start(out=iota_sb[:], in_=iota[:])
        dinvD_sb = cpool.tile([P, prm.NSW * prm.TPSW], f32, tag="dd")
        nc.sync.dma_start(out=dinvD_sb[:], in_=dinvD[:])
        # materialized iota along the dst dim, constant along the B-column
        # dim: lets the one-hot is_equal keep ALL operands' inner AP dim
        # packed (stride 1), which enables the DVE 2x_1p perf mode (the
        # broadcast-inner layout runs at half rate)
        iotaF_sb = cpool.tile([P, P, SB_MAX], bf16, tag="iof")
        nc.vector.tensor_tensor(
            out=iotaF_sb[:],
            in0=iota_sb[:, :, None].to_broadcast([P, P, SB_MAX]),
            in1=iota_sb[:, :, None].to_broadcast([P, P, SB_MAX]),
            op=mybir.AluOpType.max,
        )
        nval_sb = cpool.tile([1, len(calls)], mybir.dt.int32, tag="nv")
        nc.sync.dma_start(out=nval_sb[:], in_=nval[:])
        # rotating register pool: a single count register serializes call
        # k+1's reg_load behind call k's FULL DMA completion (register-WAR
        # tracking) — with 8, the dependency lands 8 calls back
        nregs = [nc.gpsimd.alloc_register(f"nvreg{i}") for i in range(8)]
        emitted = [0]
        rctr = [0]

        chunk_tiles = {}

        def load_chunk(ch):
            if ch >= NCH or ch in chunk_tiles:
                return
            idx_t = pidx.tile([P, ICH_MAX], mybir.dt.int16, tag="idx")
            dst_t = pidx.tile([P, SCH_MAX], bf16, tag="dst")
            i0, i1, s0, s1 = ch_i0[ch]
            nc.sync.dma_start(out=idx_t[:, : i1 - i0], in_=gidx[:, i0:i1])
            nc.sync.dma_start(out=dst_t[:, : s1 - s0], in_=dstl[:, s0:s1])
            chunk_tiles[ch] = (idx_t, dst_t)

        load_chunk(0)

        # ------- Phase A: build the table shard; per-quarter AllGather ----
        # collective q fires as soon as the 5 write groups of quarter q are
        # in TBSH, so bucket-q gathers pipeline with the rest of phase A
        for g in range(NGpc):
            xt = pa.tile([P, prm.WG], bf16, tag="xt")
            nc.sync.dma_start(
                out=xt[:], in_=xT[:, g * prm.WG : (g + 1) * prm.WG]
            )
            hps = psa.tile([P, prm.J * FOUT], f32, tag="hps")
            for j in range(prm.J):
                nc.tensor.matmul(
                    out=hps[:, j * FOUT : (j + 1) * FOUT],
                    lhsT=xt[:, j * P : (j + 1) * P],
                    rhs=wt_sb[:],
                    start=True,
                    stop=True,
                )
            tsb = pa.tile([P, prm.J, TROW], bf16, tag="tsb")
            nc.vector.tensor_tensor(
                out=tsb[:, :, :FOUT],
                in0=hps[:].rearrange("p (j f) -> p j f", f=FOUT),
                in1=dinvA_sb[:, g * prm.J : (g + 1) * prm.J][
                    :, :, None
                ].to_broadcast([P, prm.J, FOUT]),
                op=mybir.AluOpType.mult,
            )
            base = prm.WG * g
            nc.sync.dma_start(
                out=TBSH[base : base + prm.WG, :].rearrange(
                    "(p j) f -> p j f", j=prm.J
                ),
                in_=tsb[:],
            )
        # assemble the full table from all cores' shards
        nc.gpsimd.collective_compute(
            "AllGather",
            mybir.AluOpType.bypass,
            replica_groups=[list(range(prm.C))],
            ins=[TBSH[:]],
            outs=[TBLA[:]],
        )
        TBL = [
            TBLA[k * (prm.BKCAP // 2) : (k + 1) * (prm.BKCAP // 2), :]
            for k in range(prm.NBK)
        ]

        # ---------------- Phase B/L: gather + segment-sum ----------------
        def emit_call(ci, cm, tiles):
            S, SB = cm.S, cm.SB
            ch = cm.sw // CHUNK_SW
            idx_t, dst_t = chunk_tiles[ch]
            io = cm.icol - ch_i0[ch][0]
            so = cm.scol - ch_i0[ch][2]
            g_t = pg.tile([P, S_MAX, GROW], bf16, tag="g")
            # slots skipped by the runtime count must hold finite data
            # (0 * NaN would poison the PE accumulation): zero each pool
            # buffer on its first use; afterwards stale rows are old table
            # data, which is finite
            if emitted[0] < prm.GBUFS:
                nc.vector.memset(g_t[:], 0.0)
            emitted[0] += 1
            nreg = nregs[rctr[0] % len(nregs)]
            rctr[0] += 1
            nc.gpsimd.reg_load(nreg, nval_sb[0:1, ci : ci + 1])
            nc.gpsimd.dma_gather(
                out_ap=g_t[:, :S, :],
                in_ap=TBL[cm.bk],
                idxs_ap=idx_t[:, io : io + 8 * S],
                num_idxs=S * P,
                num_idxs_reg=nreg,
                elem_size=GROW,
                single_packet=False,
                queue_num=cm.bk % 4,
            )
            # transposed one-hot [slot, dst, bcol]: inner dim (bcol) packed
            # on every operand -> DVE 2x_1p
            b_t = pb.tile([P, P, SB_MAX], bf16, tag="b")
            nc.vector.tensor_tensor(
                out=b_t[:, :, :SB],
                in0=dst_t[:, so : so + SB][:, None, :].to_broadcast([P, P, SB]),
                in1=iotaF_sb[:, :, :SB],
                op=mybir.AluOpType.is_equal,
            )
            tiles[ci] = (g_t, b_t)

        def emit_mms(sw, tiles, mms_src, yout, acc_pfx=""):
            # accumulation groups must never share a PSUM bank
            # (start=True clears the whole bank) - one [P, FOUT] tile per
            # dst tile, all TPSW open concurrently in separate banks
            rows_sw = min(prm.SWD, prm.NS - sw * prm.SWD)
            nt = (rows_sw + P - 1) // P  # valid dst tiles this sw
            ysc = py.tile([P, prm.TPSW, FOUT], f32, tag="ysc")
            mms = mms_src[sw]
            psum_t = {
                t: psb.tile([P, FOUT], f32, tag=f"acc{t}", name=f"{acc_pfx}acc{t}")
                for t in range(prm.TPSW)
            }
            closed = set()
            for ci, t, sl, bcol, par, st, sp in mms:
                g_t, b_t = tiles[ci]
                nc.tensor.matmul(
                    out=psum_t[t][:],
                    lhsT=b_t[:, :, bcol],
                    rhs=g_t[:, sl, par * FOUT : (par + 1) * FOUT],
                    start=st,
                    stop=sp,
                )
                if sp and t < nt:
                    # scale by dinv[dst] right after the group closes
                    w = sw * prm.TPSW + t
                    nc.scalar.activation(
                        out=ysc[:, t, :],
                        in_=psum_t[t][:],
                        func=mybir.ActivationFunctionType.Copy,
                        scale=dinvD_sb[:, w : w + 1],
                    )
                    closed.add(t)
            assert closed == set(range(nt)), (sw, closed, nt)
            for t in range(nt):
                rt = min(P, rows_sw - t * P)
                r0 = sw * prm.SWD + t * P
                nc.scalar.dma_start(
                    out=yout[r0 : r0 + rt, :], in_=ysc[:rt, t, :]
                )

        for sw in range(prm.NSW):
            if sw % CHUNK_SW == 0:
                load_chunk(sw // CHUNK_SW + 1)  # prefetch next chunk
            tiles = {}
            for bk in range(prm.NBK):
                cicm = calls_by_swbk.get((sw, bk))
                if cicm is not None:
                    emit_call(cicm[0], cicm[1], tiles)
            emit_mms(sw, tiles, mms_by_sw, y)
        _stk.close()

    nc.compile()
    _split_sync_waits(nc)
    return nc


def _get_program_and_prep(x, edge_index, W, prm):
    inputs, calls, mms_by_sw = _host_prep(x, edge_index, W, prm)
    nc = _build_program(prm, calls, mms_by_sw)
    return nc, inputs


def kernel(x, edge_index, W):
    prm = Prm(N=int(x.shape[0]))
    nc, inputs = _get_program_and_prep(x, edge_index, W, prm)
    res = run_bass_kernel_spmd(nc, inputs, list(range(prm.C)))
    y = np.concatenate([res.results[c]["y"] for c in range(prm.C)], axis=0)
    return y.astype(np.float32)


def run_with_trace(x, edge_index, W, trace_cores=None):
    """test.py helper: returns (y, BassKernelResults) with profiling."""
    prm = Prm(N=int(x.shape[0]))
    nc, inputs = _get_program_and_prep(x, edge_index, W, prm)
    res = run_bass_kernel_spmd(
        nc, inputs, list(range(prm.C)), trace=True, trace_cores=trace_cores
    )
    y = np.concatenate([res.results[c]["y"] for c in range(prm.C)], axis=0)
    return y.astype(np.float32), res
